# revision 33
# baseline (speedup 1.0000x reference)
"""MinkowskiFlow coarse-flow kernel for 8 Trainium2 NeuronCores (Bass/Tile).

Math (per batch b):
    fs = normalize(feat_s); ft = normalize(feat_t)
    C[n,m]   = 2 - 2 <fs_n, ft_m>
    K[n,m]   = exp(-C/(exp(eps)+0.03)) * (||coor_s_n - coor_t_m||^2 < 100)
    out[n,:] = (K @ coor_t) / (sum_m K + 1e-8) - coor_s

Sharding: batch b -> 4 cores each (data parallel over B=2), N split into 4
row blocks of 1024 (row-wise; each row's normalization is independent).

Per-core layout (all "transposed": target index m on SBUF partitions):
  S^T[m,n]   ONE bf16 PE matmul over the 64-dim hi parts of the normalized
             features (drops hi*lo cross terms ~3e-4 in C -> ~2% relative
             per-pair weight noise, which averages out over each row's
             ~300 support points; measured output rel err 1.7e-4 vs the
             2e-2 gate).
  dist mask  R'[m,n] = -2*ctc_m . csc_n + |csc_n|^2 computed as ONE K=21
             bf16 matmul over a 3-way bf16 split (h+m+l) of CENTERED
             coords: terms h.h, cs2(h,m,l), h.m, m.h, m.m, h.l, l.h.
             Knife-edge radius pairs carry up to 31% of a row's softmax
             weight with a 6.6e-4 margin; this split keeps |err| ~ 1e-4.
             mask = R' < 100 - |ctc_m|^2   (fp32 threshold per partition)
  K^T        = exp((2 S^T - 2)/tau) * mask, one ACT Exp + one fused DVE
             scalar_tensor_tensor (is_lt, mult; Pool has no PSUM access),
             stored bf16.
  agg        ONE bf16 matmul per tile: lhsT [128,4] holds [ct | 1] ->
             PSUM [4, n] (x, y, z, rowsum).

Pipelining:
  - agg matmul for m-tile t issued LAG tiles late so the PE never
    head-of-line blocks on the ACT->STT mask pipeline.
  - the outer repeat loop is software-pipelined: repeat i+1's preamble
    (input loads, normalize, coord splits, DMA transposes) is EMITTED in
    the middle of repeat i's main loop, so on every engine queue it
    executes while repeat i is still streaming -- no inter-repeat bubble.
    Matmul operands are double-buffered (ping-pong by repeat parity).
  - all per-tile xbar transposes are batched into ONE DmaTransposeAnt
    per operand via a 3D out AP (out[q,t,p] = in[p, t*128+q]): HWDGE
    dispatch is ~625ns PER INSTRUCTION and serializes, so 88 per-tile
    transposes would cost ~57us/repeat.
  - outputs are staged in SBUF and written by ONE deferred DMA emitted
    during the NEXT repeat (keeps the SP DMA queue from blocking the next
    repeat's loads/transposes behind an end-of-repeat dependency).
  - only Exp/Ln ACT funcs are used (one table set: 1/sqrt(x) is
    exp(-0.5 ln x)) so there is no per-repeat ACT table reload; all
    SBUF-only elementwise prep runs on the otherwise-idle Pool engine.
Final per n-tile: PE-transpose agg slice, out = acc*recip(rs+1e-8) - cs.
"""
import numpy as np
from contextlib import ExitStack

import concourse.bass as bass
import concourse.bacc as bacc
import concourse.tile as tile
import concourse.mybir as mybir
from concourse import masks

F32 = mybir.dt.float32
BF16 = mybir.dt.bfloat16
AF = mybir.ActivationFunctionType
ALU = mybir.AluOpType

B, N, M, D = 2, 4096, 4096, 64
N_CORES = 8
CORES_PER_BATCH = N_CORES // B      # 4
NS = N // CORES_PER_BATCH           # 1024 source rows per core
P = 128
MT = M // P                         # 32 target tiles
NT = NS // P                        # 8 source tiles per core
CHUNK = 512
NCHUNK = NS // CHUNK                # 2
CENTER = 20.0
TAU_OFFSET = 0.03
RADIUS_SQ = 100.0
KC = 21                             # coord-matmul contraction rows
LAG = 3                             # agg matmul issue lag (tiles)


def build_kernel(tau: float, repeat: int = 1):
    nc = bacc.Bacc("TRN2", target_bir_lowering=False, debug=False,
                   num_devices=N_CORES)
    fs_d = nc.dram_tensor("fs", [NS, D], F32, kind="ExternalInput").ap()
    ft_d = nc.dram_tensor("ft", [M, D], F32, kind="ExternalInput").ap()
    cs_d = nc.dram_tensor("cs", [NS, 3], F32, kind="ExternalInput").ap()
    ct_d = nc.dram_tensor("ct", [M, 3], F32, kind="ExternalInput").ap()
    out_d = nc.dram_tensor("out", [NS, 3], F32, kind="ExternalOutput").ap()

    scale = float(2.0 / tau)

    with tile.TileContext(nc) as tc, ExitStack() as ctx:
        pers = ctx.enter_context(tc.tile_pool(name="pers", bufs=1))
        dbuf = ctx.enter_context(tc.tile_pool(name="dbuf", bufs=2))
        sbE = ctx.enter_context(tc.tile_pool(name="sbE", bufs=4))
        sbK = ctx.enter_context(tc.tile_pool(name="sbK", bufs=6))
        fin = ctx.enter_context(tc.tile_pool(name="fin", bufs=2))
        psA = ctx.enter_context(tc.tile_pool(name="psA", bufs=3, space="PSUM"))
        psB = ctx.enter_context(tc.tile_pool(name="psB", bufs=3, space="PSUM"))
        psG = ctx.enter_context(tc.tile_pool(name="psG", bufs=1, space="PSUM"))
        psS = ctx.enter_context(tc.tile_pool(name="psS", bufs=1, space="PSUM"))

        # ---------------- persistent staging / scratch ----------------
        ident = pers.tile([P, P], F32)
        biasT = pers.tile([P, 1], F32)

        ft_all = pers.tile([P, MT * D], F32)
        fs_all = pers.tile([P, NT * D], F32)
        ct_all = pers.tile([P, MT * 3], F32)
        sqf_t = pers.tile([P, MT * D], F32)
        sqf_s = pers.tile([P, NT * D], F32)
        s2t = pers.tile([P, MT], F32)
        s2s = pers.tile([P, NT], F32)
        ln_t = pers.tile([P, MT], F32)
        ln_s = pers.tile([P, NT], F32)
        rn_t = pers.tile([P, MT], F32)
        rn_s = pers.tile([P, NT], F32)
        fhT_st = pers.tile([P, MT * P], BF16)  # per tile: [hi(64) | 0(64)]
        fsT_st = pers.tile([P, NT * P], BF16)  # per tile: [hi | 0]
        # coord splits (target / source), 3 cols per tile
        ctn_all = pers.tile([P, MT * 3], F32)
        th_all = pers.tile([P, MT * 3], BF16)
        tm_all = pers.tile([P, MT * 3], BF16)
        tl_all = pers.tile([P, MT * 3], BF16)
        tr1 = pers.tile([P, MT * 3], F32)
        sq_t = pers.tile([P, MT * 3], F32)
        sq_s = pers.tile([P, NT * 3], F32)
        ct2c = pers.tile([P, MT], F32)
        cs2c = pers.tile([P, NT], F32)
        csc_all = pers.tile([P, NT * 3], F32)
        sh_all = pers.tile([P, NT * 3], BF16)
        sm_all = pers.tile([P, NT * 3], BF16)
        sl_all = pers.tile([P, NT * 3], BF16)
        sr1 = pers.tile([P, NT * 3], F32)
        c2h = pers.tile([P, NT], BF16)
        c2m = pers.tile([P, NT], BF16)
        c2l = pers.tile([P, NT], BF16)
        c2r = pers.tile([P, NT], F32)
        rbt = pers.tile([P, MT * P], BF16)   # row-layout coord lhsT builder
        rbs = pers.tile([P, NT * P], BF16)   # (cols KC:128 zero-padded)

        masks.make_identity(nc, ident[:])
        nc.vector.memset(biasT[:], -scale)
        # structural zeros written ONCE (per-rep writes only touch the
        # nonzero columns)
        nc.vector.memset(fhT_st[:], 0.0)
        nc.vector.memset(fsT_st[:], 0.0)
        nc.vector.memset(rbt[:], 0.0)
        nc.vector.memset(rbs[:], 0.0)

        def emit_preamble():
            """Loads + operand prep for one repeat. Returns the operand
            tile handles (double-buffered; ping-pong by call parity)."""
            ftT = dbuf.tile([P, M], BF16, tag="ftT")     # rows 0:64 ft_hi^T
            rhsA = dbuf.tile([P, NS], BF16, tag="rhsA")  # rows 0:64 fs_hi^T
            lhsC = dbuf.tile([P, M], BF16, tag="lhsC")   # rows 0:KC coords^T
            rhsC = dbuf.tile([P, NS], BF16, tag="rhsC")  # rows 0:KC coords
            thr = dbuf.tile([P, MT], F32, tag="thr")     # 100 - |ct-20|^2
            ct4 = dbuf.tile([P, 4 * MT], BF16, tag="ct4")
            # cs is read by the finishing block at the END of a repeat;
            # double-buffered so the next repeat's load can't stall the
            # SP DMA queue (and every transpose behind it)
            cs_all = dbuf.tile([P, NT * 3], F32, tag="cs")

            nc.sync.dma_start(
                ft_all[:].rearrange("p (t d) -> p t d", d=D),
                ft_d.rearrange("(t p) d -> p t d", p=P))
            nc.sync.dma_start(
                fs_all[:].rearrange("p (t d) -> p t d", d=D),
                fs_d.rearrange("(t p) d -> p t d", p=P))
            nc.sync.dma_start(
                ct_all[:].rearrange("p (t c) -> p t c", c=3),
                ct_d.rearrange("(t p) c -> p t c", p=P))
            nc.sync.dma_start(
                cs_all[:].rearrange("p (t c) -> p t c", c=3),
                cs_d.rearrange("(t p) c -> p t c", p=P))

            ftv = ft_all[:].rearrange("p (t d) -> p t d", d=D)
            fsv = fs_all[:].rearrange("p (t d) -> p t d", d=D)

            # ------- feature normalization (hi parts only) ----------
            # Square stays on ACT (shares the Exp table set); 1/sqrt on DVE.
            nc.gpsimd.tensor_tensor(sqf_t[:], ft_all[:], ft_all[:],
                                    op=ALU.mult)
            nc.vector.tensor_reduce(
                s2t[:], sqf_t[:].rearrange("p (t d) -> p t d", d=D),
                axis=mybir.AxisListType.X, op=ALU.add)
            nc.gpsimd.tensor_tensor(sqf_s[:], fs_all[:], fs_all[:],
                                    op=ALU.mult)
            nc.vector.tensor_reduce(
                s2s[:], sqf_s[:].rearrange("p (t d) -> p t d", d=D),
                axis=mybir.AxisListType.X, op=ALU.add)
            # 1/sqrt(x) = exp(-0.5 ln x): Ln/Exp/Square share ONE ACT table
            # set (natural_log_exp_and_others) -> no per-repeat table reload
            nc.scalar.activation(ln_t[:], s2t[:], AF.Ln)
            nc.scalar.activation(rn_t[:], ln_t[:], AF.Exp, scale=-0.5)
            nc.scalar.activation(ln_s[:], s2s[:], AF.Ln)
            nc.scalar.activation(rn_s[:], ln_s[:], AF.Exp, scale=-0.5)

            # normalize via stride-0 broadcast of 1/|f| (Pool has no
            # scalar-ptr ops; TensorScalarPtr fails the engine ISA check)
            vhT = fhT_st[:].rearrange("p (t k) -> p t k", k=P)
            nc.gpsimd.tensor_tensor(vhT[:, :, 0:D], ftv[:],
                                    rn_t[:].broadcast_to([P, MT, D]),
                                    op=ALU.mult)
            vsT = fsT_st[:].rearrange("p (t k) -> p t k", k=P)
            nc.gpsimd.tensor_tensor(vsT[:, :, 0:D], fsv[:],
                                    rn_s[:].broadcast_to([P, NT, D]),
                                    op=ALU.mult)
            # ONE batched xbar transpose per operand (3D out AP = per-tile
            # 128x128 transposes): HWDGE dispatch is ~625ns PER INSTRUCTION
            # and serializes, so 88 per-tile transposes would cost ~57us
            nc.sync.dma_start_transpose(
                rhsA[:].rearrange("p (t k) -> p t k", k=P), fsT_st[:])
            nc.sync.dma_start_transpose(
                ftT[:].rearrange("p (t k) -> p t k", k=P), fhT_st[:])

            # ---------------- coordinates ----------------
            # target: ctn = -2*(ct-20) = -2*ct + 40, 3-way bf16 split
            nc.gpsimd.tensor_scalar(ctn_all[:], ct_all[:], -2.0, 2.0 * CENTER,
                                    op0=ALU.mult, op1=ALU.add)
            nc.gpsimd.tensor_copy(th_all[:], ctn_all[:])
            nc.gpsimd.tensor_tensor(tr1[:], ctn_all[:], th_all[:],
                                    op=ALU.subtract)
            nc.gpsimd.tensor_copy(tm_all[:], tr1[:])
            nc.gpsimd.tensor_tensor(tl_all[:], tr1[:], tm_all[:],
                                    op=ALU.subtract)
            # |ct-20|^2 = |ctn|^2 / 4 ; thr = 100 - |ct-20|^2
            nc.gpsimd.tensor_tensor(sq_t[:], ctn_all[:], ctn_all[:],
                                    op=ALU.mult)
            nc.vector.tensor_reduce(
                ct2c[:], sq_t[:].rearrange("p (t c) -> p t c", c=3),
                axis=mybir.AxisListType.X, op=ALU.add)
            nc.gpsimd.tensor_scalar(thr[:], ct2c[:], -0.25, RADIUS_SQ,
                                    op0=ALU.mult, op1=ALU.add)
            # agg lhsT: [ct | 1] in bf16 (bf16 coord rounding adds ~3e-3
            # rel err vs the 2e-2 gate; dropping the hi/lo split halves the
            # agg output rows and removes the hi+lo add entirely)
            v4 = ct4[:].rearrange("p (t k) -> p t k", k=4)
            vctv = ct_all[:].rearrange("p (t c) -> p t c", c=3)
            nc.gpsimd.tensor_copy(v4[:, :, 0:3], vctv[:])
            nc.gpsimd.memset(v4[:, :, 3:4], 1.0)
            # coord lhsT row-layout builder: [h, 1, h, m, m, h, l] then T
            rt = rbt[:].rearrange("p (t k) -> p t k", k=P)
            vth = th_all[:].rearrange("p (t c) -> p t c", c=3)
            vtm = tm_all[:].rearrange("p (t c) -> p t c", c=3)
            vtl = tl_all[:].rearrange("p (t c) -> p t c", c=3)
            nc.gpsimd.tensor_copy(rt[:, :, 0:3], vth[:])
            nc.gpsimd.memset(rt[:, :, 3:6], 1.0)
            nc.gpsimd.tensor_copy(rt[:, :, 6:9], vth[:])
            nc.gpsimd.tensor_copy(rt[:, :, 9:12], vtm[:])
            nc.gpsimd.tensor_copy(rt[:, :, 12:15], vtm[:])
            nc.gpsimd.tensor_copy(rt[:, :, 15:18], vth[:])
            nc.gpsimd.tensor_copy(rt[:, :, 18:21], vtl[:])
            nc.sync.dma_start_transpose(
                lhsC[:].rearrange("p (t k) -> p t k", k=P), rbt[:])

            # source: csc = cs - 20, 3-way split; cs2 = |csc|^2, 3-way split
            nc.gpsimd.tensor_scalar_add(csc_all[:], cs_all[:], -CENTER)
            nc.gpsimd.tensor_copy(sh_all[:], csc_all[:])
            nc.gpsimd.tensor_tensor(sr1[:], csc_all[:], sh_all[:],
                                    op=ALU.subtract)
            nc.gpsimd.tensor_copy(sm_all[:], sr1[:])
            nc.gpsimd.tensor_tensor(sl_all[:], sr1[:], sm_all[:],
                                    op=ALU.subtract)
            nc.gpsimd.tensor_tensor(sq_s[:], csc_all[:], csc_all[:],
                                    op=ALU.mult)
            nc.vector.tensor_reduce(
                cs2c[:], sq_s[:].rearrange("p (t c) -> p t c", c=3),
                axis=mybir.AxisListType.X, op=ALU.add)
            nc.gpsimd.tensor_copy(c2h[:], cs2c[:])
            nc.gpsimd.tensor_tensor(c2r[:], cs2c[:], c2h[:], op=ALU.subtract)
            nc.gpsimd.tensor_copy(c2m[:], c2r[:])
            nc.gpsimd.tensor_tensor(c2l[:], c2r[:], c2m[:], op=ALU.subtract)
            # source rows: [h, cs2h, cs2m, cs2l, m, h, m, l, h]
            rs_ = rbs[:].rearrange("p (t k) -> p t k", k=P)
            vsh = sh_all[:].rearrange("p (t c) -> p t c", c=3)
            vsm = sm_all[:].rearrange("p (t c) -> p t c", c=3)
            vsl = sl_all[:].rearrange("p (t c) -> p t c", c=3)
            rs2 = rbs[:].rearrange("p (t k) -> p k t", k=P)
            nc.gpsimd.tensor_copy(rs_[:, :, 0:3], vsh[:])
            nc.gpsimd.tensor_copy(rs2[:, 3, :], c2h[:])
            nc.gpsimd.tensor_copy(rs2[:, 4, :], c2m[:])
            nc.gpsimd.tensor_copy(rs2[:, 5, :], c2l[:])
            nc.gpsimd.tensor_copy(rs_[:, :, 6:9], vsm[:])
            nc.gpsimd.tensor_copy(rs_[:, :, 9:12], vsh[:])
            nc.gpsimd.tensor_copy(rs_[:, :, 12:15], vsm[:])
            nc.gpsimd.tensor_copy(rs_[:, :, 15:18], vsl[:])
            nc.gpsimd.tensor_copy(rs_[:, :, 18:21], vsh[:])
            nc.sync.dma_start_transpose(
                rhsC[:].rearrange("p (t k) -> p t k", k=P), rbs[:])

            return {"ftT": ftT, "rhsA": rhsA, "lhsC": lhsC, "rhsC": rhsC,
                    "thr": thr, "ct4": ct4, "cs": cs_all}

        def emit_chunk(op, j):
            """Main loop for chunk j using operand handles `op`."""
            ftT, rhsA = op["ftT"], op["rhsA"]
            lhsC, rhsC = op["lhsC"], op["rhsC"]
            thr, ct4 = op["thr"], op["ct4"]
            cols = slice(j * CHUNK, (j + 1) * CHUNK)
            aggp = psG.tile([4, CHUNK], F32, tag="agg")
            ks = [None] * MT
            for mt in range(MT + LAG):
                if mt < MT:
                    msl = slice(mt * P, (mt + 1) * P)
                    sp = psA.tile([P, CHUNK], F32, tag="sp")
                    nc.tensor.matmul(sp[:], ftT[0:D, msl], rhsA[0:D, cols],
                                     start=True, stop=True)
                    rp = psB.tile([P, CHUNK], F32, tag="rp")
                    nc.tensor.matmul(rp[:], lhsC[0:KC, msl], rhsC[0:KC, cols],
                                     start=True, stop=True)
                    e = sbE.tile([P, CHUNK], BF16, tag="e")
                    nc.scalar.activation(e[:], sp[:], AF.Exp,
                                         bias=biasT[:], scale=scale)
                    k = sbK.tile([P, CHUNK], BF16, tag="k")
                    # Pool/GPSIMD cannot access PSUM (rp) -> DVE only
                    nc.vector.scalar_tensor_tensor(k[:], in0=rp[:],
                                             scalar=thr[:, mt:mt + 1],
                                             in1=e[:], op0=ALU.is_lt,
                                             op1=ALU.mult)
                    ks[mt] = k
                if mt >= LAG:
                    mta = mt - LAG
                    nc.tensor.matmul(aggp[:], ct4[:, 4 * mta:4 * mta + 4],
                                     ks[mta][:], start=(mta == 0),
                                     stop=(mta == MT - 1))
            return aggp

        def emit_finish(op, j, aggp, res_all):
            """Per-chunk epilogue: rowsum-normalized coords minus coor_s,
            staged into res_all (written to DRAM by a deferred DMA)."""
            csv = op["cs"][:].rearrange("p (t c) -> p t c", c=3)
            rv = res_all[:].rearrange("p (t c) -> p t c", c=3)
            agg_sb = fin.tile([4, CHUNK], F32, tag="aggsb")
            nc.vector.tensor_copy(agg_sb[:], aggp[0:4, :])
            for tl in range(CHUNK // P):
                nt = j * (CHUNK // P) + tl
                tp = psS.tile([P, 4], F32, tag="tp")
                nc.tensor.matmul(tp[:], agg_sb[:, tl * P:(tl + 1) * P],
                                 ident[0:4, 0:4], is_transpose=True)
                tsb = fin.tile([P, 4], F32, tag="tsb")
                nc.vector.tensor_copy(tsb[:], tp[:])
                rec = fin.tile([P, 1], F32, tag="rec")
                nc.vector.tensor_scalar_add(rec[:], tsb[:, 3:4], 1e-8)
                nc.vector.reciprocal(rec[:], rec[:])
                nc.vector.scalar_tensor_tensor(rv[:, nt, :], in0=tsb[:, 0:3],
                                               scalar=rec[:], in1=csv[:, nt, :],
                                               op0=ALU.mult, op1=ALU.subtract)

        def emit_out_dma(res_all):
            nc.sync.dma_start(out_d.rearrange("(t p) c -> p t c", p=P),
                              res_all[:].rearrange("p (t c) -> p t c", c=3))

        # ---- software-pipelined outer loop ----
        op = emit_preamble()
        pending_res = None
        for rep in range(repeat):
            res_all = dbuf.tile([P, NT * 3], F32, tag="res")
            a0 = emit_chunk(op, 0)
            if pending_res is not None:
                emit_out_dma(pending_res)
            next_op = emit_preamble() if rep + 1 < repeat else None
            emit_finish(op, 0, a0, res_all)
            a1 = emit_chunk(op, 1)
            emit_finish(op, 1, a1, res_all)
            pending_res = res_all
            if next_op is not None:
                op = next_op
        emit_out_dma(pending_res)

    nc.compile()
    return nc


_CACHE = {}


def make_sharded_exec(nc):
    """One-time: wrap the compiled Bass module as a cached 8-device sharded
    PJRT executable (avoids run_bass_kernel_spmd's per-call re-trace)."""
    import jax
    from jax.sharding import Mesh, PartitionSpec, NamedSharding
    from jax.experimental.shard_map import shard_map
    from concourse import bass2jax
    bass2jax.install_neuronx_cc_hook()

    part_name = (nc.partition_id_tensor.name
                 if nc.partition_id_tensor else None)
    in_names, out_names, out_avals, zero_outs = [], [], [], []
    for alloc in nc.m.functions[0].allocations:
        if not isinstance(alloc, mybir.MemoryLocationSet):
            continue
        name = alloc.memorylocations[0].name
        if alloc.kind == "ExternalInput":
            if name != part_name:
                in_names.append(name)
        elif alloc.kind == "ExternalOutput":
            out_names.append(name)
            shape = tuple(alloc.tensor_shape)
            dtype = mybir.dt.np(alloc.dtype)
            out_avals.append(jax.core.ShapedArray(shape, dtype))
            zero_outs.append(np.zeros(shape, dtype))
    n_params = len(in_names)
    all_names = in_names + out_names
    if part_name is not None:
        all_names = all_names + [part_name]

    def _body(*args):
        operands = list(args)
        if part_name is not None:
            operands.append(bass2jax.partition_id_tensor())
        outs = bass2jax._bass_exec_p.bind(
            *operands, out_avals=tuple(out_avals), in_names=tuple(all_names),
            out_names=tuple(out_names), lowering_input_output_aliases=(),
            sim_require_finite=True, sim_require_nnan=True, nc=nc)
        return tuple(outs)

    devices = jax.devices()[:N_CORES]
    mesh = Mesh(np.asarray(devices), ("core",))
    in_specs = (PartitionSpec("core"),) * (n_params + len(out_names))
    out_specs = (PartitionSpec("core"),) * len(out_names)
    sharded = jax.jit(shard_map(_body, mesh=mesh, in_specs=in_specs,
                                out_specs=out_specs, check_rep=False),
                      keep_unused=True)
    sh = NamedSharding(mesh, PartitionSpec("core"))
    return {"sharded": sharded, "in_names": in_names,
            "out_names": out_names, "zero_outs": zero_outs, "sh": sh}


def shard_inputs(feat_s, feat_t, coor_s, coor_t):
    """Per-core input blocks -> concatenated global arrays, keyed by name."""
    per = {"fs": [], "ft": [], "cs": [], "ct": []}
    for c in range(N_CORES):
        b = c // CORES_PER_BATCH
        r = c % CORES_PER_BATCH
        sl = slice(r * NS, (r + 1) * NS)
        per["fs"].append(feat_s[b, sl])
        per["ft"].append(feat_t[b])
        per["cs"].append(coor_s[b, sl])
        per["ct"].append(coor_t[b])
    return {k: np.ascontiguousarray(np.concatenate(v, axis=0))
            for k, v in per.items()}


def run_sharded(ex, glb):
    import jax
    dev_args = [jax.device_put(glb[n], ex["sh"]) for n in ex["in_names"]] + [
        jax.device_put(np.concatenate([z] * N_CORES, axis=0), ex["sh"])
        for z in ex["zero_outs"]]
    outs = ex["sharded"](*dev_args)
    got = np.asarray(outs[ex["out_names"].index("out")]).reshape(N_CORES, NS, 3)
    out = np.empty((B, N, 3), dtype=np.float32)
    for c in range(N_CORES):
        b = c // CORES_PER_BATCH
        r = c % CORES_PER_BATCH
        out[b, r * NS:(r + 1) * NS] = got[c]
    return out


def kernel(feat_s, feat_t, coor_s, coor_t, epsilon):
    feat_s = np.ascontiguousarray(feat_s, dtype=np.float32)
    feat_t = np.ascontiguousarray(feat_t, dtype=np.float32)
    coor_s = np.ascontiguousarray(coor_s, dtype=np.float32)
    coor_t = np.ascontiguousarray(coor_t, dtype=np.float32)
    tau = float(np.exp(np.float32(epsilon)) + np.float32(TAU_OFFSET))

    key = round(tau, 12)
    if key not in _CACHE:
        nc = build_kernel(tau)
        _CACHE[key] = (nc, make_sharded_exec(nc))
    nc, ex = _CACHE[key]
    return run_sharded(ex, shard_inputs(feat_s, feat_t, coor_s, coor_t))


# revision 39
# speedup vs baseline: 1.0103x; 1.0103x over previous
"""MinkowskiFlow coarse-flow kernel for 8 Trainium2 NeuronCores (Bass/Tile).

Math (per batch b):
    fs = normalize(feat_s); ft = normalize(feat_t)
    C[n,m]   = 2 - 2 <fs_n, ft_m>
    K[n,m]   = exp(-C/(exp(eps)+0.03)) * (||coor_s_n - coor_t_m||^2 < 100)
    out[n,:] = (K @ coor_t) / (sum_m K + 1e-8) - coor_s

Sharding: batch b -> 4 cores each (data parallel over B=2), N split into 4
row blocks of 1024 (row-wise; each row's normalization is independent).

Per-core layout (all "transposed": target index m on SBUF partitions):
  S^T[m,n]   ONE bf16 PE matmul over the 64-dim hi parts of the normalized
             features (drops hi*lo cross terms ~3e-4 in C -> ~2% relative
             per-pair weight noise, which averages out over each row's
             ~300 support points; measured output rel err 1.7e-4 vs the
             2e-2 gate).
  dist mask  R'[m,n] = -2*ctc_m . csc_n + |csc_n|^2 computed as ONE K=21
             bf16 matmul over a 3-way bf16 split (h+m+l) of CENTERED
             coords: terms h.h, cs2(h,m,l), h.m, m.h, m.m, h.l, l.h.
             Knife-edge radius pairs carry up to 31% of a row's softmax
             weight with a 6.6e-4 margin; this split keeps |err| ~ 1e-4.
             mask = R' < 100 - |ctc_m|^2   (fp32 threshold per partition)
  K^T        = exp((2 S^T - 2)/tau) * mask, one ACT Exp + one fused DVE
             scalar_tensor_tensor (is_lt, mult; Pool has no PSUM access),
             stored bf16.
  agg        ONE bf16 matmul per tile: lhsT [128,4] holds [ct | 1] ->
             PSUM [4, n] (x, y, z, rowsum).

Pipelining:
  - agg matmul for m-tile t issued LAG tiles late so the PE never
    head-of-line blocks on the ACT->STT mask pipeline.
  - the outer repeat loop is software-pipelined: repeat i+1's preamble
    (input loads, normalize, coord splits, DMA transposes) is EMITTED in
    the middle of repeat i's main loop, so on every engine queue it
    executes while repeat i is still streaming -- no inter-repeat bubble.
    Matmul operands are double-buffered (ping-pong by repeat parity).
  - all per-tile xbar transposes are batched into ONE DmaTransposeAnt
    per operand via a 3D out AP (out[q,t,p] = in[p, t*128+q]): HWDGE
    dispatch is ~625ns PER INSTRUCTION and serializes, so 88 per-tile
    transposes would cost ~57us/repeat.
  - outputs are staged in SBUF and written by ONE deferred DMA emitted
    during the NEXT repeat (keeps the SP DMA queue from blocking the next
    repeat's loads/transposes behind an end-of-repeat dependency).
  - the ONLY ACT func is Exp (no table reloads) and the ONLY DVE work
    is the mask: the whole preamble chain (squares, tree-reduced norms,
    Quake-rsqrt with 2 Newton steps, splits, builders) runs on the
    otherwise-idle Pool engine, so at repeat boundaries the saturated
    DVE/ACT queues never wait on next-repeat prep.
Final per n-tile: PE-transpose agg slice, out = acc*recip(rs+1e-8) - cs.
"""
import numpy as np
from contextlib import ExitStack

import concourse.bass as bass
import concourse.bacc as bacc
import concourse.tile as tile
import concourse.mybir as mybir
from concourse import masks

F32 = mybir.dt.float32
BF16 = mybir.dt.bfloat16
AF = mybir.ActivationFunctionType
ALU = mybir.AluOpType

B, N, M, D = 2, 4096, 4096, 64
N_CORES = 8
CORES_PER_BATCH = N_CORES // B      # 4
NS = N // CORES_PER_BATCH           # 1024 source rows per core
P = 128
MT = M // P                         # 32 target tiles
NT = NS // P                        # 8 source tiles per core
CHUNK = 512
NCHUNK = NS // CHUNK                # 2
CENTER = 20.0
TAU_OFFSET = 0.03
RADIUS_SQ = 100.0
KC = 21                             # coord-matmul contraction rows
LAG = 3                             # agg matmul issue lag (tiles)


def build_kernel(tau: float, repeat: int = 1):
    nc = bacc.Bacc("TRN2", target_bir_lowering=False, debug=False,
                   num_devices=N_CORES)
    fs_d = nc.dram_tensor("fs", [NS, D], F32, kind="ExternalInput").ap()
    ft_d = nc.dram_tensor("ft", [M, D], F32, kind="ExternalInput").ap()
    cs_d = nc.dram_tensor("cs", [NS, 3], F32, kind="ExternalInput").ap()
    ct_d = nc.dram_tensor("ct", [M, 3], F32, kind="ExternalInput").ap()
    out_d = nc.dram_tensor("out", [NS, 3], F32, kind="ExternalOutput").ap()

    scale = float(2.0 / tau)

    with tile.TileContext(nc) as tc, ExitStack() as ctx:
        pers = ctx.enter_context(tc.tile_pool(name="pers", bufs=1))
        dbuf = ctx.enter_context(tc.tile_pool(name="dbuf", bufs=2))
        sbE = ctx.enter_context(tc.tile_pool(name="sbE", bufs=4))
        sbK = ctx.enter_context(tc.tile_pool(name="sbK", bufs=6))
        fin = ctx.enter_context(tc.tile_pool(name="fin", bufs=2))
        psA = ctx.enter_context(tc.tile_pool(name="psA", bufs=3, space="PSUM"))
        psB = ctx.enter_context(tc.tile_pool(name="psB", bufs=3, space="PSUM"))
        psG = ctx.enter_context(tc.tile_pool(name="psG", bufs=1, space="PSUM"))
        psS = ctx.enter_context(tc.tile_pool(name="psS", bufs=1, space="PSUM"))

        # ---------------- persistent staging / scratch ----------------
        ident = pers.tile([P, P], F32)
        biasT = pers.tile([P, 1], F32)

        ft_all = pers.tile([P, MT * D], F32)
        fs_all = pers.tile([P, NT * D], F32)
        ct_all = pers.tile([P, MT * 3], F32)
        sqf_t = pers.tile([P, MT * D], F32)
        sqf_s = pers.tile([P, NT * D], F32)
        s2t = pers.tile([P, MT], F32)
        s2s = pers.tile([P, NT], F32)
        rn_t = pers.tile([P, MT], F32)
        rn_s = pers.tile([P, NT], F32)
        qi_t = pers.tile([P, MT], mybir.dt.int32)
        qi_s = pers.tile([P, NT], mybir.dt.int32)
        nw_t = pers.tile([P, MT], F32)
        nw_s = pers.tile([P, NT], F32)
        fhT_st = pers.tile([P, MT * P], BF16)  # per tile: [hi(64) | 0(64)]
        fsT_st = pers.tile([P, NT * P], BF16)  # per tile: [hi | 0]
        # coord splits (target / source), 3 cols per tile
        ctn_all = pers.tile([P, MT * 3], F32)
        th_all = pers.tile([P, MT * 3], BF16)
        tm_all = pers.tile([P, MT * 3], BF16)
        tl_all = pers.tile([P, MT * 3], BF16)
        tr1 = pers.tile([P, MT * 3], F32)
        sq_t = pers.tile([P, MT * 3], F32)
        sq_s = pers.tile([P, NT * 3], F32)
        ct2c = pers.tile([P, MT], F32)
        cs2c = pers.tile([P, NT], F32)
        csc_all = pers.tile([P, NT * 3], F32)
        sh_all = pers.tile([P, NT * 3], BF16)
        sm_all = pers.tile([P, NT * 3], BF16)
        sl_all = pers.tile([P, NT * 3], BF16)
        sr1 = pers.tile([P, NT * 3], F32)
        c2h = pers.tile([P, NT], BF16)
        c2m = pers.tile([P, NT], BF16)
        c2l = pers.tile([P, NT], BF16)
        c2r = pers.tile([P, NT], F32)
        rbt = pers.tile([P, MT * P], BF16)   # row-layout coord lhsT builder
        rbs = pers.tile([P, NT * P], BF16)   # (cols KC:128 zero-padded)

        masks.make_identity(nc, ident[:])
        nc.vector.memset(biasT[:], -scale)
        # structural zeros written ONCE (per-rep writes only touch the
        # nonzero columns)
        nc.vector.memset(fhT_st[:], 0.0)
        nc.vector.memset(fsT_st[:], 0.0)
        nc.vector.memset(rbt[:], 0.0)
        nc.vector.memset(rbs[:], 0.0)

        def pool_sumsq(sq, s2, ntiles, width):
            """s2[p,t] = sum over the width-sized last axis of sq (viewed
            [p, t, width]) via in-place halving adds — Pool only, so the
            next repeat's norm chain never queues behind the saturated
            DVE/ACT streams."""
            v = sq[:].rearrange("p (t d) -> p t d", d=width)
            w = width // 2
            while w >= 1:
                nc.gpsimd.tensor_tensor(v[:, :, 0:w], v[:, :, 0:w],
                                        v[:, :, w:2 * w], op=ALU.add)
                w //= 2
            if width == 3:
                nc.gpsimd.tensor_tensor(v[:, :, 0:1], v[:, :, 0:1],
                                        v[:, :, 2:3], op=ALU.add)
            col0 = sq[:].rearrange("p (t d) -> p d t", d=width)[:, 0, :]
            nc.gpsimd.tensor_copy(s2[:], col0)

        def pool_rsqrt(s2, rn, qi, nw):
            """rn = 1/sqrt(s2) on Pool: Quake seed (bitcast + shift +
            bitwise-not trick for C - (i>>1)) + 2 Newton steps."""
            # int seed ops on DVE (Pool's ISA rejects int shift); they are
            # 3 tiny [128,32] ops mid-queue, not at the repeat boundary
            nc.vector.tensor_scalar(qi[:], s2[:].bitcast(mybir.dt.int32), 1,
                                    None, op0=ALU.arith_shift_right)
            # C - h == (~h) + (C+1)  (two's complement; avoids int mult;
            # bitwise and arith ALU ops cannot share one instruction)
            nc.vector.tensor_scalar(qi[:], qi[:], 0, None,
                                    op0=ALU.bitwise_not)
            nc.vector.tensor_scalar(rn[:].bitcast(mybir.dt.int32), qi[:],
                                    0x5f3759df + 1, None, op0=ALU.add)
            for _ in range(2):
                nc.gpsimd.tensor_tensor(nw[:], rn[:], rn[:], op=ALU.mult)
                nc.gpsimd.tensor_tensor(nw[:], nw[:], s2[:], op=ALU.mult)
                nc.gpsimd.tensor_scalar(nw[:], nw[:], -0.5, 1.5,
                                        op0=ALU.mult, op1=ALU.add)
                nc.gpsimd.tensor_tensor(rn[:], rn[:], nw[:], op=ALU.mult)

        def emit_preamble():
            """Loads + operand prep for one repeat. Returns the operand
            tile handles (double-buffered; ping-pong by call parity)."""
            ftT = dbuf.tile([P, M], BF16, tag="ftT")     # rows 0:64 ft_hi^T
            rhsA = dbuf.tile([P, NS], BF16, tag="rhsA")  # rows 0:64 fs_hi^T
            lhsC = dbuf.tile([P, M], BF16, tag="lhsC")   # rows 0:KC coords^T
            rhsC = dbuf.tile([P, NS], BF16, tag="rhsC")  # rows 0:KC coords
            thr = dbuf.tile([P, MT], F32, tag="thr")     # 100 - |ct-20|^2
            ct4 = dbuf.tile([P, 4 * MT], BF16, tag="ct4")
            # cs is read by the finishing block at the END of a repeat;
            # double-buffered so the next repeat's load can't stall the
            # SP DMA queue (and every transpose behind it)
            cs_all = dbuf.tile([P, NT * 3], F32, tag="cs")

            nc.sync.dma_start(
                ft_all[:].rearrange("p (t d) -> p t d", d=D),
                ft_d.rearrange("(t p) d -> p t d", p=P))
            nc.sync.dma_start(
                fs_all[:].rearrange("p (t d) -> p t d", d=D),
                fs_d.rearrange("(t p) d -> p t d", p=P))
            nc.sync.dma_start(
                ct_all[:].rearrange("p (t c) -> p t c", c=3),
                ct_d.rearrange("(t p) c -> p t c", p=P))
            nc.sync.dma_start(
                cs_all[:].rearrange("p (t c) -> p t c", c=3),
                cs_d.rearrange("(t p) c -> p t c", p=P))

            ftv = ft_all[:].rearrange("p (t d) -> p t d", d=D)
            fsv = fs_all[:].rearrange("p (t d) -> p t d", d=D)

            # ------- feature normalization (hi parts only) ----------
            # Square stays on ACT (shares the Exp table set); 1/sqrt on DVE.
            nc.gpsimd.tensor_tensor(sqf_t[:], ft_all[:], ft_all[:],
                                    op=ALU.mult)
            pool_sumsq(sqf_t, s2t, MT, D)
            nc.gpsimd.tensor_tensor(sqf_s[:], fs_all[:], fs_all[:],
                                    op=ALU.mult)
            pool_sumsq(sqf_s, s2s, NT, D)
            pool_rsqrt(s2t, rn_t, qi_t, nw_t)
            pool_rsqrt(s2s, rn_s, qi_s, nw_s)

            # normalize via stride-0 broadcast of 1/|f| (Pool has no
            # scalar-ptr ops; TensorScalarPtr fails the engine ISA check)
            vhT = fhT_st[:].rearrange("p (t k) -> p t k", k=P)
            nc.gpsimd.tensor_tensor(vhT[:, :, 0:D], ftv[:],
                                    rn_t[:].broadcast_to([P, MT, D]),
                                    op=ALU.mult)
            vsT = fsT_st[:].rearrange("p (t k) -> p t k", k=P)
            nc.gpsimd.tensor_tensor(vsT[:, :, 0:D], fsv[:],
                                    rn_s[:].broadcast_to([P, NT, D]),
                                    op=ALU.mult)
            # ONE batched xbar transpose per operand (3D out AP = per-tile
            # 128x128 transposes): HWDGE dispatch is ~625ns PER INSTRUCTION
            # and serializes, so 88 per-tile transposes would cost ~57us
            nc.sync.dma_start_transpose(
                rhsA[:].rearrange("p (t k) -> p t k", k=P), fsT_st[:])
            nc.sync.dma_start_transpose(
                ftT[:].rearrange("p (t k) -> p t k", k=P), fhT_st[:])

            # ---------------- coordinates ----------------
            # target: ctn = -2*(ct-20) = -2*ct + 40, 3-way bf16 split
            nc.gpsimd.tensor_scalar(ctn_all[:], ct_all[:], -2.0, 2.0 * CENTER,
                                    op0=ALU.mult, op1=ALU.add)
            nc.gpsimd.tensor_copy(th_all[:], ctn_all[:])
            nc.gpsimd.tensor_tensor(tr1[:], ctn_all[:], th_all[:],
                                    op=ALU.subtract)
            nc.gpsimd.tensor_copy(tm_all[:], tr1[:])
            nc.gpsimd.tensor_tensor(tl_all[:], tr1[:], tm_all[:],
                                    op=ALU.subtract)
            # |ct-20|^2 = |ctn|^2 / 4 ; thr = 100 - |ct-20|^2
            nc.gpsimd.tensor_tensor(sq_t[:], ctn_all[:], ctn_all[:],
                                    op=ALU.mult)
            pool_sumsq(sq_t, ct2c, MT, 3)
            nc.gpsimd.tensor_scalar(thr[:], ct2c[:], -0.25, RADIUS_SQ,
                                    op0=ALU.mult, op1=ALU.add)
            # agg lhsT: [ct | 1] in bf16 (bf16 coord rounding adds ~3e-3
            # rel err vs the 2e-2 gate; dropping the hi/lo split halves the
            # agg output rows and removes the hi+lo add entirely)
            v4 = ct4[:].rearrange("p (t k) -> p t k", k=4)
            vctv = ct_all[:].rearrange("p (t c) -> p t c", c=3)
            nc.gpsimd.tensor_copy(v4[:, :, 0:3], vctv[:])
            nc.gpsimd.memset(v4[:, :, 3:4], 1.0)
            # coord lhsT row-layout builder: [h, 1, h, m, m, h, l] then T
            rt = rbt[:].rearrange("p (t k) -> p t k", k=P)
            vth = th_all[:].rearrange("p (t c) -> p t c", c=3)
            vtm = tm_all[:].rearrange("p (t c) -> p t c", c=3)
            vtl = tl_all[:].rearrange("p (t c) -> p t c", c=3)
            nc.gpsimd.tensor_copy(rt[:, :, 0:3], vth[:])
            nc.gpsimd.memset(rt[:, :, 3:6], 1.0)
            nc.gpsimd.tensor_copy(rt[:, :, 6:9], vth[:])
            nc.gpsimd.tensor_copy(rt[:, :, 9:12], vtm[:])
            nc.gpsimd.tensor_copy(rt[:, :, 12:15], vtm[:])
            nc.gpsimd.tensor_copy(rt[:, :, 15:18], vth[:])
            nc.gpsimd.tensor_copy(rt[:, :, 18:21], vtl[:])
            nc.sync.dma_start_transpose(
                lhsC[:].rearrange("p (t k) -> p t k", k=P), rbt[:])

            # source: csc = cs - 20, 3-way split; cs2 = |csc|^2, 3-way split
            nc.gpsimd.tensor_scalar_add(csc_all[:], cs_all[:], -CENTER)
            nc.gpsimd.tensor_copy(sh_all[:], csc_all[:])
            nc.gpsimd.tensor_tensor(sr1[:], csc_all[:], sh_all[:],
                                    op=ALU.subtract)
            nc.gpsimd.tensor_copy(sm_all[:], sr1[:])
            nc.gpsimd.tensor_tensor(sl_all[:], sr1[:], sm_all[:],
                                    op=ALU.subtract)
            nc.gpsimd.tensor_tensor(sq_s[:], csc_all[:], csc_all[:],
                                    op=ALU.mult)
            pool_sumsq(sq_s, cs2c, NT, 3)
            nc.gpsimd.tensor_copy(c2h[:], cs2c[:])
            nc.gpsimd.tensor_tensor(c2r[:], cs2c[:], c2h[:], op=ALU.subtract)
            nc.gpsimd.tensor_copy(c2m[:], c2r[:])
            nc.gpsimd.tensor_tensor(c2l[:], c2r[:], c2m[:], op=ALU.subtract)
            # source rows: [h, cs2h, cs2m, cs2l, m, h, m, l, h]
            rs_ = rbs[:].rearrange("p (t k) -> p t k", k=P)
            vsh = sh_all[:].rearrange("p (t c) -> p t c", c=3)
            vsm = sm_all[:].rearrange("p (t c) -> p t c", c=3)
            vsl = sl_all[:].rearrange("p (t c) -> p t c", c=3)
            rs2 = rbs[:].rearrange("p (t k) -> p k t", k=P)
            nc.gpsimd.tensor_copy(rs_[:, :, 0:3], vsh[:])
            nc.gpsimd.tensor_copy(rs2[:, 3, :], c2h[:])
            nc.gpsimd.tensor_copy(rs2[:, 4, :], c2m[:])
            nc.gpsimd.tensor_copy(rs2[:, 5, :], c2l[:])
            nc.gpsimd.tensor_copy(rs_[:, :, 6:9], vsm[:])
            nc.gpsimd.tensor_copy(rs_[:, :, 9:12], vsh[:])
            nc.gpsimd.tensor_copy(rs_[:, :, 12:15], vsm[:])
            nc.gpsimd.tensor_copy(rs_[:, :, 15:18], vsl[:])
            nc.gpsimd.tensor_copy(rs_[:, :, 18:21], vsh[:])
            nc.sync.dma_start_transpose(
                rhsC[:].rearrange("p (t k) -> p t k", k=P), rbs[:])

            return {"ftT": ftT, "rhsA": rhsA, "lhsC": lhsC, "rhsC": rhsC,
                    "thr": thr, "ct4": ct4, "cs": cs_all}

        def emit_chunk(op, j):
            """Main loop for chunk j using operand handles `op`."""
            ftT, rhsA = op["ftT"], op["rhsA"]
            lhsC, rhsC = op["lhsC"], op["rhsC"]
            thr, ct4 = op["thr"], op["ct4"]
            cols = slice(j * CHUNK, (j + 1) * CHUNK)
            aggp = psG.tile([4, CHUNK], F32, tag="agg")
            ks = [None] * MT
            for mt in range(MT + LAG):
                if mt < MT:
                    msl = slice(mt * P, (mt + 1) * P)
                    sp = psA.tile([P, CHUNK], F32, tag="sp")
                    nc.tensor.matmul(sp[:], ftT[0:D, msl], rhsA[0:D, cols],
                                     start=True, stop=True)
                    rp = psB.tile([P, CHUNK], F32, tag="rp")
                    nc.tensor.matmul(rp[:], lhsC[0:KC, msl], rhsC[0:KC, cols],
                                     start=True, stop=True)
                    e = sbE.tile([P, CHUNK], BF16, tag="e")
                    nc.scalar.activation(e[:], sp[:], AF.Exp,
                                         bias=biasT[:], scale=scale)
                    k = sbK.tile([P, CHUNK], BF16, tag="k")
                    # Pool/GPSIMD cannot access PSUM (rp) -> DVE only
                    nc.vector.scalar_tensor_tensor(k[:], in0=rp[:],
                                             scalar=thr[:, mt:mt + 1],
                                             in1=e[:], op0=ALU.is_lt,
                                             op1=ALU.mult)
                    ks[mt] = k
                if mt >= LAG:
                    mta = mt - LAG
                    nc.tensor.matmul(aggp[:], ct4[:, 4 * mta:4 * mta + 4],
                                     ks[mta][:], start=(mta == 0),
                                     stop=(mta == MT - 1))
            return aggp

        def emit_finish(op, j, aggp, res_all):
            """Per-chunk epilogue: rowsum-normalized coords minus coor_s,
            staged into res_all (written to DRAM by a deferred DMA)."""
            csv = op["cs"][:].rearrange("p (t c) -> p t c", c=3)
            rv = res_all[:].rearrange("p (t c) -> p t c", c=3)
            agg_sb = fin.tile([4, CHUNK], F32, tag="aggsb")
            nc.vector.tensor_copy(agg_sb[:], aggp[0:4, :])
            for tl in range(CHUNK // P):
                nt = j * (CHUNK // P) + tl
                tp = psS.tile([P, 4], F32, tag="tp")
                nc.tensor.matmul(tp[:], agg_sb[:, tl * P:(tl + 1) * P],
                                 ident[0:4, 0:4], is_transpose=True)
                tsb = fin.tile([P, 4], F32, tag="tsb")
                nc.vector.tensor_copy(tsb[:], tp[:])
                rec = fin.tile([P, 1], F32, tag="rec")
                nc.vector.tensor_scalar_add(rec[:], tsb[:, 3:4], 1e-8)
                nc.vector.reciprocal(rec[:], rec[:])
                nc.vector.scalar_tensor_tensor(rv[:, nt, :], in0=tsb[:, 0:3],
                                               scalar=rec[:], in1=csv[:, nt, :],
                                               op0=ALU.mult, op1=ALU.subtract)

        def emit_out_dma(res_all):
            nc.sync.dma_start(out_d.rearrange("(t p) c -> p t c", p=P),
                              res_all[:].rearrange("p (t c) -> p t c", c=3))

        # ---- software-pipelined outer loop ----
        op = emit_preamble()
        pending_res = None
        for rep in range(repeat):
            res_all = dbuf.tile([P, NT * 3], F32, tag="res")
            a0 = emit_chunk(op, 0)
            if pending_res is not None:
                emit_out_dma(pending_res)
            next_op = emit_preamble() if rep + 1 < repeat else None
            emit_finish(op, 0, a0, res_all)
            a1 = emit_chunk(op, 1)
            emit_finish(op, 1, a1, res_all)
            pending_res = res_all
            if next_op is not None:
                op = next_op
        emit_out_dma(pending_res)

    nc.compile()
    return nc


_CACHE = {}


def make_sharded_exec(nc):
    """One-time: wrap the compiled Bass module as a cached 8-device sharded
    PJRT executable (avoids run_bass_kernel_spmd's per-call re-trace)."""
    import jax
    from jax.sharding import Mesh, PartitionSpec, NamedSharding
    from jax.experimental.shard_map import shard_map
    from concourse import bass2jax
    bass2jax.install_neuronx_cc_hook()

    part_name = (nc.partition_id_tensor.name
                 if nc.partition_id_tensor else None)
    in_names, out_names, out_avals, zero_outs = [], [], [], []
    for alloc in nc.m.functions[0].allocations:
        if not isinstance(alloc, mybir.MemoryLocationSet):
            continue
        name = alloc.memorylocations[0].name
        if alloc.kind == "ExternalInput":
            if name != part_name:
                in_names.append(name)
        elif alloc.kind == "ExternalOutput":
            out_names.append(name)
            shape = tuple(alloc.tensor_shape)
            dtype = mybir.dt.np(alloc.dtype)
            out_avals.append(jax.core.ShapedArray(shape, dtype))
            zero_outs.append(np.zeros(shape, dtype))
    n_params = len(in_names)
    all_names = in_names + out_names
    if part_name is not None:
        all_names = all_names + [part_name]

    def _body(*args):
        operands = list(args)
        if part_name is not None:
            operands.append(bass2jax.partition_id_tensor())
        outs = bass2jax._bass_exec_p.bind(
            *operands, out_avals=tuple(out_avals), in_names=tuple(all_names),
            out_names=tuple(out_names), lowering_input_output_aliases=(),
            sim_require_finite=True, sim_require_nnan=True, nc=nc)
        return tuple(outs)

    devices = jax.devices()[:N_CORES]
    mesh = Mesh(np.asarray(devices), ("core",))
    in_specs = (PartitionSpec("core"),) * (n_params + len(out_names))
    out_specs = (PartitionSpec("core"),) * len(out_names)
    sharded = jax.jit(shard_map(_body, mesh=mesh, in_specs=in_specs,
                                out_specs=out_specs, check_rep=False),
                      keep_unused=True)
    sh = NamedSharding(mesh, PartitionSpec("core"))
    return {"sharded": sharded, "in_names": in_names,
            "out_names": out_names, "zero_outs": zero_outs, "sh": sh}


def shard_inputs(feat_s, feat_t, coor_s, coor_t):
    """Per-core input blocks -> concatenated global arrays, keyed by name."""
    per = {"fs": [], "ft": [], "cs": [], "ct": []}
    for c in range(N_CORES):
        b = c // CORES_PER_BATCH
        r = c % CORES_PER_BATCH
        sl = slice(r * NS, (r + 1) * NS)
        per["fs"].append(feat_s[b, sl])
        per["ft"].append(feat_t[b])
        per["cs"].append(coor_s[b, sl])
        per["ct"].append(coor_t[b])
    return {k: np.ascontiguousarray(np.concatenate(v, axis=0))
            for k, v in per.items()}


def run_sharded(ex, glb):
    import jax
    dev_args = [jax.device_put(glb[n], ex["sh"]) for n in ex["in_names"]] + [
        jax.device_put(np.concatenate([z] * N_CORES, axis=0), ex["sh"])
        for z in ex["zero_outs"]]
    outs = ex["sharded"](*dev_args)
    got = np.asarray(outs[ex["out_names"].index("out")]).reshape(N_CORES, NS, 3)
    out = np.empty((B, N, 3), dtype=np.float32)
    for c in range(N_CORES):
        b = c // CORES_PER_BATCH
        r = c % CORES_PER_BATCH
        out[b, r * NS:(r + 1) * NS] = got[c]
    return out


def kernel(feat_s, feat_t, coor_s, coor_t, epsilon):
    feat_s = np.ascontiguousarray(feat_s, dtype=np.float32)
    feat_t = np.ascontiguousarray(feat_t, dtype=np.float32)
    coor_s = np.ascontiguousarray(coor_s, dtype=np.float32)
    coor_t = np.ascontiguousarray(coor_t, dtype=np.float32)
    tau = float(np.exp(np.float32(epsilon)) + np.float32(TAU_OFFSET))

    key = round(tau, 12)
    if key not in _CACHE:
        nc = build_kernel(tau)
        _CACHE[key] = (nc, make_sharded_exec(nc))
    nc, ex = _CACHE[key]
    return run_sharded(ex, shard_inputs(feat_s, feat_t, coor_s, coor_t))
